# revision 1
# baseline (speedup 1.0000x reference)
"""GCN block kernel for TRN2, 8-core SPMD.

Algorithm (per core, destination-sharded):
  out[dst] = relu( (sum_e norm_e * x[src_e]) @ W.T + bias ) + x[dst]
Reassociation: the edge aggregation runs on raw x rows (gathered via
dma_gather), accumulated per 128-row destination tile via one-hot matmuls
on the PE; a single 128x128 weight matmul per destination tile finishes
the job, with bias folded in as a rank-1 matmul.

Edge partitioning: by destination core/tile, sub-grouped by source window
(int16 gather index limit = 32768 rows). Uniform SPMD schedule: per-
(tile,window) groups padded to a shared capacity; trailing -1 indices are
skipped by the Q7 descriptor generator at ~zero cost.
"""
import sys
sys.path.insert(0, '/opt/trn_rl_repo')
import numpy as np
from contextlib import ExitStack

import concourse.bacc as bacc
import concourse.mybir as mybir
from concourse.library_config import mlp

F32 = mybir.dt.float32
I16 = mybir.dt.int16
ALU = mybir.AluOpType

D = 128


def roundup(x, m):
    return (x + m - 1) // m * m


class Cfg:
    def __init__(self, N, E, NC=8, WIN=32768, TB=8, NG=6, NP=32, NF=4, NEO=4):
        self.N, self.E, self.NC = N, E, NC
        assert N % NC == 0
        self.SHARD = N // NC
        self.TILES = (self.SHARD + 127) // 128  # dst tiles per core
        self.WIN = WIN
        self.NW = (N + WIN - 1) // WIN          # source windows
        self.TB = TB                            # dst tiles per batch
        self.NG = NG                            # gather ring depth
        self.NP = NP                            # P ring (chunks)
        self.NF = NF                            # final psum ring
        self.NEO = NEO                          # epilogue sbuf ring


def prep(cfg, x, weight, bias, edge_weight, edge_index):
    """Host preprocessing -> (schedule, per-core in_maps pieces, caps)."""
    N, NC, SHARD, TILES, WIN, NW = cfg.N, cfg.NC, cfg.SHARD, cfg.TILES, cfg.WIN, cfg.NW
    src = np.asarray(edge_index[0], dtype=np.int64)
    dst = np.asarray(edge_index[1], dtype=np.int64)
    ew = np.asarray(edge_weight, dtype=np.float64)
    deg = np.bincount(dst, weights=ew, minlength=N) + 1.0
    dinv = (1.0 / np.sqrt(deg)).astype(np.float32)
    norm = (dinv[src] * ew.astype(np.float32) * dinv[dst]).astype(np.float32)

    loop = np.arange(N, dtype=np.int64)
    a_src = np.concatenate([src, loop])
    a_dst = np.concatenate([dst, loop])
    a_nrm = np.concatenate([norm, (dinv * dinv).astype(np.float32)])

    core = a_dst // SHARD
    dstloc = a_dst - core * SHARD
    tile = dstloc // 128
    col = (dstloc % 128).astype(np.float32)
    win = a_src // WIN
    srcloc = (a_src - win * WIN).astype(np.int32)

    key = (core * TILES + tile) * NW + win
    order = np.argsort(key, kind='stable')
    k_sorted = key[order]
    counts = np.bincount(key, minlength=NC * TILES * NW).reshape(NC, TILES, NW)
    starts = np.zeros(NC * TILES * NW + 1, dtype=np.int64)
    np.cumsum(counts.reshape(-1), out=starts[1:])

    cap_w = [max(128, roundup(int(counts[:, :, w].max()), 128)) for w in range(NW)]

    s_srcloc = srcloc[order]
    s_nrm = a_nrm[order]
    s_col = col[order]

    # Call order: tile-major (each tile's NW windows consecutive) so PSUM
    # accumulation groups stay contiguous on the PE.
    n_batches = TILES
    sched = []   # per call dict
    for I in range(TILES):
        for w in range(NW):
            sched.append(dict(w=w, I=I, slot=I % 2,
                              first=(w == 0), last=(w == NW - 1)))

    # uniform valid count per call across cores: num_idxs_reg must equal the
    # actual non-negative count on every core (one SPMD program, one immediate)
    vq = []
    for q, call in enumerate(sched):
        mx = 1
        for c in range(NC):
            g = (c * TILES + call['I']) * NW + call['w']
            mx = max(mx, int(starts[g + 1] - starts[g]))
        vq.append(mx)

    # per-call capacity: vq rounded to a chunk — DVE/PE only touch real chunks
    for q, call in enumerate(sched):
        call['cap'] = roundup(vq[q], 128)
        call['CH'] = call['cap'] // 128
    icols = [c['cap'] // 16 for c in sched]
    chs = [c['CH'] for c in sched]
    icol_off = np.concatenate([[0], np.cumsum(icols)])
    ch_off = np.concatenate([[0], np.cumsum(chs)])
    ICOLS_TOT, CH_TOT = int(icol_off[-1]), int(ch_off[-1])

    idx_streams, nv_streams, dv_streams, counts_per_call = [], [], [], []
    for c in range(NC):
        idx_s = np.full((16, ICOLS_TOT), -1, dtype=np.int16)
        nd_s = np.zeros((128, 2 * CH_TOT), dtype=np.float32)
        ccnt = []
        for q, call in enumerate(sched):
            g = (c * TILES + call['I']) * NW + call['w']
            lo, hi = int(starts[g]), int(starts[g + 1])
            cnt = hi - lo
            cap = call['cap']
            assert cnt <= cap, (cnt, cap)
            if cnt == 0:
                # one harmless dummy so the gather isn't empty
                iv = np.zeros(1, dtype=np.int16)
                nv = np.zeros(1, dtype=np.float32)
                dvv = np.zeros(1, dtype=np.float32)
                cnt = 1
            else:
                iv = s_srcloc[lo:hi].astype(np.int16)
                nv = s_nrm[lo:hi]
                dvv = s_col[lo:hi]
            ccnt.append(cnt)
            ipad = np.full(cap, -1, dtype=np.int16)
            ipad[:vq[q]] = 0
            ipad[:cnt] = iv
            idx_s[:, icol_off[q]:icol_off[q + 1]] = ipad.reshape(-1, 16).T
            npad = np.zeros(cap, dtype=np.float32); npad[:cnt] = nv
            dpad = np.zeros(cap, dtype=np.float32); dpad[:cnt] = dvv
            ch = cap // 128
            nd_s[:, 2 * ch_off[q]:2 * ch_off[q] + ch] = npad.reshape(-1, 128).T
            nd_s[:, 2 * ch_off[q] + ch:2 * ch_off[q + 1]] = dpad.reshape(-1, 128).T
        idx_streams.append(np.tile(idx_s, (8, 1)))
        nv_streams.append(nd_s)
        counts_per_call.append(ccnt)

    meta = dict(sched=sched, cap_w=cap_w, icol_off=icol_off, ch_off=ch_off, vq=vq,
                ICOLS_TOT=ICOLS_TOT, CH_TOT=CH_TOT, n_batches=n_batches,
                counts_per_call=counts_per_call)

    wt = np.ascontiguousarray(np.asarray(weight, dtype=np.float32).T)  # wt[k,o]=W[o,k]
    bias_row = np.asarray(bias, dtype=np.float32).reshape(1, D)
    ones_row = np.ones((1, D), dtype=np.float32)
    iota = np.tile(np.arange(D, dtype=np.float32), (128, 1))

    xf = np.asarray(x, dtype=np.float32)
    in_maps = []
    for c in range(NC):
        in_maps.append({
            "xfull": xf,
            "xshard": np.ascontiguousarray(xf[c * SHARD:(c + 1) * SHARD]),
            "idxs": idx_streams[c],
            "nds": nv_streams[c],
            "wt": wt, "bias_row": bias_row, "ones_row": ones_row, "iota": iota,
        })
    return meta, in_maps


def build(cfg, meta, sim_core=None, strip=None, reps=1):
    """Build the SPMD program. sim_core: if set, use that core's exact
    per-call counts as num_idxs_reg (CoreSim validation)."""
    N, SHARD, TILES, WIN, NW = cfg.N, cfg.SHARD, cfg.TILES, cfg.WIN, cfg.NW
    sched, icol_off, ch_off = meta['sched'], meta['icol_off'], meta['ch_off']
    NCALLS = len(sched)
    CHmax = max(c['CH'] for c in sched)
    ICOLmax = max(c['cap'] // 16 for c in sched)

    # Flatten reps x sched into one global schedule with global call index q,
    # global tile index t, and dram_q for DRAM stream offsets.
    gsched = []
    for r in range(reps):
        for q, call in enumerate(sched):
            gsched.append(dict(call, dram_q=q, t=r * TILES + call['I']))
    GT = reps * TILES   # total global tiles
    cum_chunks = np.concatenate([[0], np.cumsum([c['CH'] for c in gsched])])
    tile_last_chunk = {}
    for q, call in enumerate(gsched):
        if call['last']:
            tile_last_chunk[call['t']] = int(cum_chunks[q + 1]) - 1

    nc = bacc.Bacc("TRN2", num_swdge_queues=4)

    xfull = nc.dram_tensor("xfull", [N, D], F32, kind="ExternalInput")
    xshard = nc.dram_tensor("xshard", [SHARD, D], F32, kind="ExternalInput")
    idxs_d = nc.dram_tensor("idxs", [128, meta['ICOLS_TOT']], I16, kind="ExternalInput")
    nds_d = nc.dram_tensor("nds", [128, 2 * meta['CH_TOT']], F32, kind="ExternalInput")
    wt_d = nc.dram_tensor("wt", [D, D], F32, kind="ExternalInput")
    bias_d = nc.dram_tensor("bias_row", [1, D], F32, kind="ExternalInput")
    ones_d = nc.dram_tensor("ones_row", [1, D], F32, kind="ExternalInput")
    iota_d = nc.dram_tensor("iota", [128, D], F32, kind="ExternalInput")
    out_d = nc.dram_tensor("out", [SHARD, D], F32, kind="ExternalOutput")

    NG, NP, NF, NEO, TB = cfg.NG, cfg.NP, cfg.NF, cfg.NEO, cfg.TB

    st = ExitStack()
    gS = [st.enter_context(nc.sbuf_tensor(f"g{k}", [128, CHmax, D], F32)) for k in range(NG)]
    iS = [st.enter_context(nc.sbuf_tensor(f"ix{k}", [128, ICOLmax], I16)) for k in range(NG)]
    ndS = [st.enter_context(nc.sbuf_tensor(f"nd{k}", [128, 2 * CHmax], F32)) for k in range(NG)]
    pS = st.enter_context(nc.sbuf_tensor("pring", [128, NP * 128], F32))
    zS = st.enter_context(nc.sbuf_tensor("zring", [128, 2 * 128], F32))
    eoS = [st.enter_context(nc.sbuf_tensor(f"eo{k}", [128, D], F32)) for k in range(NEO)]
    xrS = [st.enter_context(nc.sbuf_tensor(f"xr{k}", [128, D], F32)) for k in range(NEO)]
    wtS = st.enter_context(nc.sbuf_tensor("wts", [D, D], F32))
    biasS = st.enter_context(nc.sbuf_tensor("biass", [1, D], F32))
    onesS = st.enter_context(nc.sbuf_tensor("oness", [1, D], F32))
    iotaS = st.enter_context(nc.sbuf_tensor("iotas", [128, D], F32))

    accum = st.enter_context(nc.psum_tensor("accum", [128, 2 * 512], F32))
    finalP = st.enter_context(nc.psum_tensor("finalp", [128, NF * 512], F32))

    s_idx = [st.enter_context(nc.semaphore(f"s_idx{k}")) for k in range(NG)]
    s_nd = [st.enter_context(nc.semaphore(f"s_nd{k}")) for k in range(NG)]
    s_x = [st.enter_context(nc.semaphore(f"s_x{k}")) for k in range(NEO)]
    s_out = [st.enter_context(nc.semaphore(f"s_out{k}")) for k in range(NEO)]
    s_const = st.enter_context(nc.semaphore("s_const"))
    gsem = [st.enter_context(nc.semaphore(f"gsem{k}")) for k in range(NG)]
    d_chunk = st.enter_context(nc.semaphore("d_chunk"))
    d_z = st.enter_context(nc.semaphore("d_z"))
    d_eo = st.enter_context(nc.semaphore("d_eo"))
    p_chunk = st.enter_context(nc.semaphore("p_chunk"))
    p_final = st.enter_context(nc.semaphore("p_final"))
    d_init = st.enter_context(nc.semaphore("d_init"))

    n_batches = meta['n_batches']
    counts = meta['counts_per_call'][sim_core] if sim_core is not None else None

    # epilogue tile order: tile I done in batch order
    ep_tiles = []
    for b in range(n_batches):
        ep_tiles.extend(range(b * TB, min((b + 1) * TB, TILES)))
    assert ep_tiles == list(range(TILES))

    with nc.Block() as block:

        @block.sync
        def _(sync):
            # consts
            sync.dma_start(wtS[:, :], wt_d[:, :]).then_inc(s_const, 16)
            sync.dma_start(biasS[:, :], bias_d[:, :]).then_inc(s_const, 16)
            sync.dma_start(onesS[:, :], ones_d[:, :]).then_inc(s_const, 16)
            sync.dma_start(iotaS[:, :], iota_d[:, :]).then_inc(s_const, 16)

            def store_tile(t):
                e = t % NEO
                r0 = (t % TILES) * 128
                r1 = min(r0 + 128, SHARD)
                sync.wait_ge(d_eo, t + 1)
                sync.dma_start(out_d[r0:r1, :], eoS[e][:r1 - r0, :]).then_inc(s_out[e], 16)

            def load_xr(t):
                e = t % NEO
                r0 = (t % TILES) * 128
                r1 = min(r0 + 128, SHARD)
                if t >= NEO:
                    sync.wait_ge(d_eo, t - NEO + 1)   # xr slot free
                sync.dma_start(xrS[e][:r1 - r0, :], xshard[r0:r1, :]).then_inc(s_x[e], 16)

            for q, call in enumerate(gsched):
                m = q % NG
                t = call['t']
                if call['w'] == 0 and strip is None:
                    load_xr(t)
                    if t >= 2:
                        store_tile(t - 2)
                dq = call['dram_q']
                ic0, ic1 = int(icol_off[dq]), int(icol_off[dq + 1])
                ch0, ch1 = int(ch_off[dq]), int(ch_off[dq + 1])
                if q >= NG:
                    # idx slot reused after gather of call q-NG completed
                    sync.wait_ge(gsem[m], 16 * (q // NG))
                    # nv/dv slots reused after DVE consumed call q-NG
                    if strip != 'gather':
                        sync.wait_ge(d_chunk, int(cum_chunks[q - NG + 1]))
                sync.dma_start(iS[m][:, :ic1 - ic0], idxs_d[:, ic0:ic1]).then_inc(s_idx[m], 16)
                sync.dma_start(ndS[m][:, :2 * (ch1 - ch0)], nds_d[:, 2 * ch0:2 * ch1]).then_inc(s_nd[m], 16)
            for t in (range(max(0, GT - 2), GT) if strip is None else []):
                store_tile(t)
            for e in (range(NEO) if strip is None else []):
                uses = len([t for t in range(GT) if t % NEO == e])
                if uses:
                    sync.wait_ge(s_out[e], 16 * uses)

        @block.gpsimd
        def _(gpsimd):
            gpsimd.load_library(mlp)
            gpsimd.wait_ge(d_init, NG)
            for q, call in enumerate(gsched):
                m = q % NG
                cap, CH, w = call['cap'], call['CH'], call['w']
                gpsimd.wait_ge(s_idx[m], 16 * (q // NG + 1))
                if q >= NG:
                    if strip == 'gather':
                        gpsimd.wait_ge(gsem[m], 16 * (q // NG))
                    elif strip == 'nope':
                        gpsimd.wait_ge(d_chunk, int(cum_chunks[q - NG + 1]))
                    else:
                        gpsimd.wait_ge(p_chunk, int(cum_chunks[q - NG + 1]))
                w0 = w * WIN
                w1 = min(w0 + WIN, N)
                nreg = int(meta['vq'][call['dram_q']])
                gpsimd.dma_gather(
                    gS[m][:, :CH, :], xfull[w0:w1, :], iS[m][:, :cap // 16],
                    cap, nreg, D, single_packet=False, queue_num=m % 4,
                ).then_inc(gsem[m], 16)

        @block.vector
        def _(vector):
            for k in range(NG):
                vector.memset(gS[k][:, :, :], 0.0).then_inc(d_init, 1)
            vector.wait_ge(s_const, 64)

            def epilogue(t):
                f = t % NF
                e = t % NEO
                vector.wait_ge(p_final, t + 1)
                vector.wait_ge(s_x[e], 16 * (t // NEO + 1))
                if t >= NEO:
                    vector.wait_ge(s_out[e], 16 * (t // NEO))  # eo slot free
                vector.scalar_tensor_tensor(
                    eoS[e][:, :], finalP[:, f * 512:f * 512 + 128], 0.0,
                    xrS[e][:, :], ALU.max, ALU.add,
                ).then_inc(d_eo, 1)

            if strip == 'gather':
                return
            for q, call in enumerate(gsched):
                m = q % NG
                CH, t = call['CH'], call['t']
                vector.wait_ge(s_nd[m], 16 * (q // NG + 1))
                for j in range(CH):
                    g = int(cum_chunks[q]) + j
                    p = g % NP
                    if g >= NP and strip != 'nope':
                        vector.wait_ge(p_chunk, g - NP + 1)
                    vector.tensor_scalar(
                        pS[:, p * 128:(p + 1) * 128], iotaS[:, :],
                        ndS[m][:, CH + j:CH + j + 1], ndS[m][:, j:j + 1],
                        ALU.is_equal, ALU.mult,
                    ).then_inc(d_chunk, 1)
                if call['last'] and strip is None:
                    # copy this tile's accumulated Z out of PSUM
                    z = t % 2
                    vector.wait_ge(p_chunk, tile_last_chunk[t] + 1)
                    if t >= 2:
                        vector.wait_ge(p_final, t - 1)  # zS slot free
                    vector.tensor_copy(
                        zS[:, z * 128:(z + 1) * 128],
                        accum[:, z * 512:z * 512 + 128],
                    ).then_inc(d_z, 1)
                    if t >= 1:
                        epilogue(t - 1)
            if strip is None:
                epilogue(GT - 1)

        @block.tensor
        def _(tensor):
            if strip in ('gather', 'nope'):
                return
            tensor.wait_ge(s_const, 64)

            def finals(t):
                z = t % 2
                f = t % NF
                tensor.wait_ge(d_z, t + 1)
                if t >= NF:
                    tensor.wait_ge(d_eo, t - NF + 1)
                tensor.matmul(
                    finalP[:, f * 512:f * 512 + 128],
                    zS[:, z * 128:(z + 1) * 128], wtS[:, :],
                    start=True, stop=False, skip_group_check=True,
                )
                tensor.matmul(
                    finalP[:, f * 512:f * 512 + 128],
                    onesS[:1, :], biasS[:1, :],
                    start=False, stop=True, skip_group_check=True,
                ).then_inc(p_final, 1)

            for q, call in enumerate(gsched):
                m = q % NG
                CH, t = call['CH'], call['t']
                tensor.wait_ge(gsem[m], 16 * (q // NG + 1))
                for j in range(CH):
                    g = int(cum_chunks[q]) + j
                    p = g % NP
                    tensor.wait_ge(d_chunk, g + 1)
                    is_first = call['first'] and j == 0
                    is_last = call['last'] and j == CH - 1
                    tensor.matmul(
                        accum[:, (t % 2) * 512:(t % 2) * 512 + 128],
                        gS[m][:, j, :], pS[:, p * 128:(p + 1) * 128],
                        start=is_first, stop=is_last, skip_group_check=True,
                    ).then_inc(p_chunk, 1)
                # finals of previous tile overlap this tile's chunks
                if call['first'] and t >= 1 and strip is None:
                    finals(t - 1)
            if strip is None:
                finals(GT - 1)

    st.close()
    nc.compile()
    return nc


def reference_np(x, weight, bias, edge_weight, edge_index):
    N = x.shape[0]
    src = np.asarray(edge_index[0], dtype=np.int64)
    dst = np.asarray(edge_index[1], dtype=np.int64)
    ew = np.asarray(edge_weight, dtype=np.float64)
    deg = np.bincount(dst, weights=ew, minlength=N) + 1.0
    dinv = 1.0 / np.sqrt(deg)
    h = x.astype(np.float64) @ np.asarray(weight, dtype=np.float64).T
    nrm = dinv[src] * ew * dinv[dst]
    msg = h[src] * nrm[:, None]
    out = np.zeros_like(h)
    np.add.at(out, dst, msg)
    out += (dinv * dinv)[:, None] * h
    out = out + np.asarray(bias, dtype=np.float64)
    out = np.maximum(out, 0.0) + x.astype(np.float64)
    return out


_CFG = Cfg(100000, 3200000, WIN=32768, NG=8, NP=32, NF=4, NEO=4)


def kernel(x, weight, bias, edge_weight, edge_index):
    """GCN block on 8 Trainium2 NeuronCores. Full inputs in, full output out."""
    from concourse.bass_utils import run_bass_kernel_spmd

    x = np.ascontiguousarray(np.asarray(x, dtype=np.float32))
    weight = np.asarray(weight, dtype=np.float32)
    bias = np.asarray(bias, dtype=np.float32)
    edge_weight = np.asarray(edge_weight, dtype=np.float32)
    edge_index = np.asarray(edge_index)

    meta, in_maps = prep(_CFG, x, weight, bias, edge_weight, edge_index)
    nc = build(_CFG, meta)
    res = run_bass_kernel_spmd(nc, in_maps, list(range(_CFG.NC)))
    out = np.concatenate([res.results[c]["out"] for c in range(_CFG.NC)], axis=0)
    return out.astype(np.float32)

